# revision 46
# baseline (speedup 1.0000x reference)
"""ChebNet (K=3, 2 layers) forward on 8 Trainium2 NeuronCores.

Self-contained: hardcodes the problem shapes (50000 nodes, 800000 edges,
128-d input, 128-d hidden, 40 classes).

v4 (this version), on top of the v3 packed-gather design:
- 4 SWDGE queues (num_swdge_queues=4) with gathers round-robined across
  queue pairs: descriptor generation for the edge gathers runs on all four
  Q7 core pairs concurrently (~3x the single-queue SWDGE rate, which was
  the kernel bottleneck at ~9.7ns/idx).
- One-hot segment-sum matrices are host-precomputed in fp8(e4m3) and
  streamed from DRAM in 64-matrix blocks (matmul takes fp8 rhs against
  fp16 lhsT directly), eliminating the per-entry IS_EQ on the vector
  engine entirely.
- Phase 0 moved to the host: the phase-1 table (dinv*x, fp16, chunk-concat
  layout) and x^T are kernel inputs; the table is one DRAM copy and the
  phase-1 AllGathers disappear.
- AllGather per phase reduced to 2 chunks aligned exactly with the lo/hi
  gather halves; transposed feature maps (xT/u1T/hT/pT) and premixed
  weights are fp16, freeing SBUF for deeper gather/one-hot buffering.

Math: w_e = -dinv[src]*dinv[dst] (separable); spmv = -dinv . segsum(
gather(dinv . h)); Chebyshev signs/factors folded into premixed weights.
Table layout (chunk-concat for the AllGather): local groups split
[24, 25]; chunk j of all 8 cores concatenated at base B_j; lo half =
rows <24576 (chunk 0), hi = rest (idx offset -24576 < int16 range).
"""
import sys
sys.path.insert(0, '/opt/trn_rl_repo')
sys.path.insert(0, '/opt/trn_rl_repo/concourse')

import numpy as np

N_NODES = 50000
N_EDGES = 800000
D = 128
NCLS = 40
NCORES = 8
P = 128
CH = 7                            # dst groups per gather chunk (49 = 7*7)

NLOC = N_NODES // NCORES          # 6250
NG = (NLOC + P - 1) // P          # 49
NLOCP = NG * P                    # 6272
VPAD = NCORES * NLOCP             # 50176

CHUNK_G = [(0, 24), (24, 49)]
CHUNK_ROWS = [(g1 - g0) * P for g0, g1 in CHUNK_G]
CHUNK_B = [0, 24576]
HALF = 24576

_prog_cache = {}


def _build_program(plan, debug=False):
    """plan: dict with
       T_lo/T_hi: [n_chunks] tiles per gather chunk,
       OFF_lo/OFF_hi: gidx tile offsets per chunk,
       entries: list over chunks of list over groups-in-chunk of
                (g, [(half, tile_in_chunk, dstl_col)]) ."""
    import concourse.bacc as bacc
    import concourse.mybir as mybir
    import concourse.tile as tile
    from concourse.masks import make_identity
    from concourse.bass import _add_dep_helper

    f32 = mybir.dt.float32
    f16 = mybir.dt.float16
    i16 = mybir.dt.int16
    Act = mybir.ActivationFunctionType
    Alu = mybir.AluOpType

    T_lo, T_hi = plan["T_lo"], plan["T_hi"]
    OFF_lo, OFF_hi = plan["OFF_lo"], plan["OFF_hi"]
    entries = plan["entries"]
    NTLO, NTHI = sum(T_lo), sum(T_hi)          # total gather tiles per half
    TMAXL, TMAXH = max(T_lo), max(T_hi)

    nc = bacc.Bacc(num_devices=NCORES, debug=debug,
                   num_swdge_queues=4,
                   dynamic_dma_scratch_size=16384)

    NE_tot = plan["NE_tot"]
    NB = 64                            # one-hots per streamed block
    tbl0_in = nc.declare_dram_parameter("tbl0", [VPAD, D], f16, isOutput=False)
    xT_in = nc.declare_dram_parameter("xT", [128, NLOCP], f16, isOutput=False)
    gidx_lo_in = nc.declare_dram_parameter("gidx_lo", [128, NTLO * 8], i16, isOutput=False)
    gidx_hi_in = nc.declare_dram_parameter("gidx_hi", [128, NTHI * 8], i16, isOutput=False)
    f8 = mybir.dt.float8e4
    ohs_in = nc.declare_dram_parameter("ohs", [128, NE_tot * D], f8, isOutput=False)
    dinvb_in = nc.declare_dram_parameter("dinvb", [128, NLOCP], f32, isOutput=False)
    Acat_in = nc.declare_dram_parameter("Acat", [128, 3 * D], f16, isOutput=False)
    Bcat_in = nc.declare_dram_parameter("Bcat", [128, 3 * NCLS], f16, isOutput=False)
    b1_in = nc.declare_dram_parameter("b1", [128, 1], f32, isOutput=False)
    b2_in = nc.declare_dram_parameter("b2", [128, 1], f32, isOutput=False)
    out_par = nc.declare_dram_parameter("out", [NLOC, NCLS], f32, isOutput=True)

    tslices = [None] + [nc.dram_tensor(f"t{i}s", [NLOCP, D], f16)
                        for i in range(1, 4)]
    tables = [nc.dram_tensor(f"T{i}", [VPAD, D], f16, addr_space="Shared")
              for i in range(4)]

    rg = [list(range(NCORES))]

    with tile.TileContext(nc) as tc:
        with tc.tile_pool(name="const", bufs=1) as cpool, \
             tc.tile_pool(name="big", bufs=3) as bigpool, \
             tc.tile_pool(name="gbuf", bufs=3) as gpool, \
             tc.tile_pool(name="work", bufs=4) as wpool, \
             tc.tile_pool(name="ohp", bufs=3) as ohpool, \
             tc.tile_pool(name="pacc", bufs=2, space="PSUM") as pacc, \
             tc.tile_pool(name="pmisc", bufs=2, space="PSUM") as pmisc:

            # table0 copy first so phase-1 gathers start early
            nc.sync.dma_start(out=tables[0][:, :], in_=tbl0_in[:, :])
            ident = cpool.tile([P, P], f32)
            make_identity(nc, ident[:])
            gidx_lo = cpool.tile([128, NTLO * 8], i16)
            nc.sync.dma_start(out=gidx_lo[:], in_=gidx_lo_in[:])
            gidx_hi = cpool.tile([128, NTHI * 8], i16)
            nc.sync.dma_start(out=gidx_hi[:], in_=gidx_hi_in[:])
            dinvb = cpool.tile([128, NLOCP], f32)
            nc.sync.dma_start(out=dinvb[:], in_=dinvb_in[:])
            Acat = cpool.tile([128, 3 * D], f16)
            nc.sync.dma_start(out=Acat[:], in_=Acat_in[:])
            Bcat = cpool.tile([128, 3 * NCLS], f16)
            nc.sync.dma_start(out=Bcat[:], in_=Bcat_in[:])
            b1_t = cpool.tile([128, 1], f32)
            nc.sync.dma_start(out=b1_t[:], in_=b1_in[:])
            b2_t = cpool.tile([128, 1], f32)
            nc.sync.dma_start(out=b2_t[:], in_=b2_in[:])

            # ---------- phase 0 is host-side: table0 = dinv*x (all nodes)
            xT = bigpool.tile([128, NLOCP], f16, tag="big")
            nc.sync.dma_start(out=xT[:], in_=xT_in[:])

            ag_list = [None]
            u1T = bigpool.tile([128, NLOCP], f16, tag="big")

            def spmv_groups(table, ags):
                """Yield (g, acc_psum) per dst group, packed gathers."""
                first = [True]
                ohstate = {"blk": -1, "view": None}

                def oh_rhs(eid):
                    blk = eid // NB
                    if blk != ohstate["blk"]:
                        oht = ohpool.tile([P, NB * D], f8, tag="ohblk")
                        cols = min(NB, NE_tot - blk * NB) * D
                        nc.sync.dma_start(
                            out=oht[:, :cols],
                            in_=ohs_in[:, blk * NB * D:blk * NB * D + cols])
                        ohstate["blk"] = blk
                        ohstate["view"] = oht
                    s = eid % NB
                    return ohstate["view"][:, s * D:(s + 1) * D]

                for ci in range(len(T_lo)):
                    glo = gpool.tile([128, TMAXL * D], f16, tag="glo")
                    glo3 = glo[:].rearrange("p (t d) -> p t d", d=D)
                    gi = nc.gpsimd.dma_gather(
                        out_ap=glo3[:, :T_lo[ci], :],
                        in_ap=table[0:HALF, :],
                        idxs_ap=gidx_lo[:, OFF_lo[ci] * 8:(OFF_lo[ci] + T_lo[ci]) * 8],
                        num_idxs=T_lo[ci] * P,
                        num_idxs_reg=T_lo[ci] * P,
                        elem_size=D, single_packet=False,
                        queue_num=(2 * ci) % 4)
                    ghi = gpool.tile([128, TMAXH * D], f16, tag="ghi")
                    ghi3 = ghi[:].rearrange("p (t d) -> p t d", d=D)
                    gi2 = nc.gpsimd.dma_gather(
                        out_ap=ghi3[:, :T_hi[ci], :],
                        in_ap=table[HALF:VPAD, :],
                        idxs_ap=gidx_hi[:, OFF_hi[ci] * 8:(OFF_hi[ci] + T_hi[ci]) * 8],
                        num_idxs=T_hi[ci] * P,
                        num_idxs_reg=T_hi[ci] * P,
                        elem_size=D, single_packet=False,
                        queue_num=(2 * ci + 1) % 4)
                    if first[0] and ags is not None:
                        _add_dep_helper(gi.ins, ags[0].ins, sync=True,
                                        reason="lo gather waits for AG lo")
                        _add_dep_helper(gi2.ins, ags[1].ins, sync=True,
                                        reason="hi gather waits for AG hi")
                        first[0] = False
                    for g, ents in entries[ci]:
                        acc = pacc.tile([P, P], f32, tag="acc")
                        ne = len(ents)
                        for k, (half, t, eid) in enumerate(ents):
                            src3 = glo3 if half == 0 else ghi3
                            nc.tensor.matmul(
                                out=acc[:], lhsT=src3[:, t, :], rhs=oh_rhs(eid),
                                start=(k == 0), stop=(k == ne - 1))
                        yield g, acc

            def build_table(src_fm_ap, g, dst_slice):
                tfm = wpool.tile([P, P], f32, tag="tfm")
                nc.vector.tensor_mul(out=tfm[:], in0=src_fm_ap,
                                     in1=dinvb[:, g * P:(g + 1) * P])
                trp = pmisc.tile([P, P], f32, tag="ptr")
                nc.tensor.transpose(out=trp[:], in_=tfm[:], identity=ident[:])
                tnm = wpool.tile([P, P], f16, tag="tnm")
                nc.scalar.activation(out=tnm[:], in_=trp[:], func=Act.Copy)
                nc.sync.dma_start(out=dst_slice[g * P:(g + 1) * P, :], in_=tnm[:])

            def maybe_fire_chunk_ags(k, g, ags_accum):
                for j, (cg0, cg1) in enumerate(CHUNK_G):
                    if g == cg1 - 1:
                        ag = nc.gpsimd.collective_compute(
                            "AllGather", Alu.bypass, replica_groups=rg,
                            ins=[tslices[k][cg0 * P:cg1 * P, :]],
                            outs=[tables[k][CHUNK_B[j]:CHUNK_B[j] + 8 * CHUNK_ROWS[j], :]])
                        ags_accum.append(ag)

            # ---------- phase 1 ----------
            ags1 = []
            done1 = set()
            for g, acc in spmv_groups(tables[0], ag_list[0]):
                cols = slice(g * P, (g + 1) * P)
                nc.vector.tensor_mul(out=u1T[:, cols], in0=acc[:], in1=dinvb[:, cols])
                build_table(u1T[:, cols], g, tslices[1])
                done1.add(g)
                for j, (cg0, cg1) in enumerate(CHUNK_G):
                    if len(ags1) == j and all(x in done1 for x in range(cg0, cg1)):
                        maybe_fire_chunk_ags(1, cg1 - 1, ags1)
            ag_list.append(ags1)

            # ---------- phase 2 ----------
            hT = bigpool.tile([128, NLOCP], f16, tag="big")
            ags2 = []
            done2 = set()
            for g, acc in spmv_groups(tables[1], ag_list[1]):
                cols = slice(g * P, (g + 1) * P)
                v = wpool.tile([P, P], f16, tag="w")
                nc.vector.tensor_mul(out=v[:], in0=acc[:], in1=dinvb[:, cols])
                o1 = pmisc.tile([P, P], f32, tag="pout")
                nc.tensor.matmul(out=o1[:], lhsT=Acat[:, 0:D], rhs=xT[:, cols],
                                 start=True, stop=False)
                nc.tensor.matmul(out=o1[:], lhsT=Acat[:, D:2 * D], rhs=u1T[:, cols],
                                 start=False, stop=False)
                nc.tensor.matmul(out=o1[:], lhsT=Acat[:, 2 * D:3 * D], rhs=v[:],
                                 start=False, stop=True)
                nc.scalar.activation(out=hT[:, cols], in_=o1[:], func=Act.Relu,
                                     bias=b1_t[:, 0:1])
                build_table(hT[:, cols], g, tslices[2])
                done2.add(g)
                for j, (cg0, cg1) in enumerate(CHUNK_G):
                    if len(ags2) == j and all(x in done2 for x in range(cg0, cg1)):
                        maybe_fire_chunk_ags(2, cg1 - 1, ags2)
            ag_list.append(ags2)

            # ---------- phase 3 ----------
            pT = bigpool.tile([128, NLOCP], f16, tag="big")
            ags3 = []
            done3 = set()
            for g, acc in spmv_groups(tables[2], ag_list[2]):
                cols = slice(g * P, (g + 1) * P)
                nc.vector.tensor_mul(out=pT[:, cols], in0=acc[:], in1=dinvb[:, cols])
                build_table(pT[:, cols], g, tslices[3])
                done3.add(g)
                for j, (cg0, cg1) in enumerate(CHUNK_G):
                    if len(ags3) == j and all(x in done3 for x in range(cg0, cg1)):
                        maybe_fire_chunk_ags(3, cg1 - 1, ags3)
            ag_list.append(ags3)

            # ---------- phase 4 ----------
            # Exp runs per group (one table residency); Ln deferred to one
            # batched pass so the scalar table never thrashes Exp<->Ln.
            lg_all = cpool.tile([P, NG * NCLS], f32)
            s_all = cpool.tile([P, NG], f32)
            for g, acc in spmv_groups(tables[3], ag_list[3]):
                cols = slice(g * P, (g + 1) * P)
                q = wpool.tile([P, P], f16, tag="w")
                nc.vector.tensor_mul(out=q[:], in0=acc[:], in1=dinvb[:, cols])
                o2 = pmisc.tile([P, P], f32, tag="pout")
                nc.tensor.matmul(out=o2[:NCLS, :], lhsT=Bcat[:, 0:NCLS],
                                 rhs=hT[:, cols], start=True, stop=False)
                nc.tensor.matmul(out=o2[:NCLS, :], lhsT=Bcat[:, NCLS:2 * NCLS],
                                 rhs=pT[:, cols], start=False, stop=False)
                nc.tensor.matmul(out=o2[:NCLS, :], lhsT=Bcat[:, 2 * NCLS:3 * NCLS],
                                 rhs=q[:], start=False, stop=True)
                lgT = wpool.tile([NCLS, P], f32, tag="lgT")
                nc.vector.tensor_scalar_add(lgT[:], o2[:NCLS, :],
                                            b2_t[:NCLS, 0:1])
                lg = pmisc.tile([P, NCLS], f32, tag="plg")
                nc.tensor.transpose(out=lg[:], in_=lgT[:], identity=ident[:NCLS, :NCLS])
                e_t = wpool.tile([P, NCLS], f32, tag="e")
                nc.scalar.activation(out=e_t[:], in_=lg[:], func=Act.Exp,
                                     accum_out=s_all[:, g:g + 1])
                nc.vector.tensor_copy(out=lg_all[:, g * NCLS:(g + 1) * NCLS],
                                      in_=lg[:])
            ls_all = cpool.tile([P, NG], f32)
            nc.scalar.activation(out=ls_all[:], in_=s_all[:], func=Act.Ln)
            for g in range(NG):
                fin = wpool.tile([P, NCLS], f32, tag="fin")
                nc.vector.tensor_scalar_sub(
                    fin[:], lg_all[:, g * NCLS:(g + 1) * NCLS],
                    ls_all[:, g:g + 1])
                rows = min(P, NLOC - g * P)
                nc.sync.dma_start(out=out_par[g * P:g * P + rows, :],
                                  in_=fin[:rows, :])

    nc.finalize()
    return nc


def _host_prep(x, edge_index, W1, b1, W2, b2):
    x = np.asarray(x, dtype=np.float32)
    ei = np.asarray(edge_index)
    W1 = np.asarray(W1, dtype=np.float32)
    b1 = np.asarray(b1, dtype=np.float32)
    W2 = np.asarray(W2, dtype=np.float32)
    b2 = np.asarray(b2, dtype=np.float32)
    src = ei[0].astype(np.int64)
    dst = ei[1].astype(np.int64)

    deg = np.bincount(src, minlength=N_NODES).astype(np.float32)
    dinv = np.where(deg > 0, 1.0 / np.sqrt(np.maximum(deg, 1e-12)), 0.0).astype(np.float32)

    # chunk-concat table row of each node
    score = src // NLOC
    slocal = src % NLOC
    sg = slocal // P
    chunk_of_g = np.zeros(NG, np.int64)
    for j, (g0, g1) in enumerate(CHUNK_G):
        chunk_of_g[g0:g1] = j
    g0_of_chunk = np.array([g0 for g0, _ in CHUNK_G], np.int64)
    rows_of_chunk = np.array(CHUNK_ROWS, np.int64)
    b_of_chunk = np.array(CHUNK_B, np.int64)
    sj = chunk_of_g[sg]
    srow_all = (b_of_chunk[sj] + score * rows_of_chunk[sj]
                + (slocal - g0_of_chunk[sj] * P))

    core = dst // NLOC
    ed = dst - core * NLOC
    grp = ed // P
    dl = (ed % P).astype(np.int16)
    is_hi = srow_all >= HALF

    # per-core sorted buckets: key = (group, half)
    buckets = []     # buckets[c] = (srow_sorted, dstl_sorted, counts[2*NG])
    for c in range(NCORES):
        m = core == c
        key = grp[m] * 2 + is_hi[m]
        order = np.argsort(key, kind='stable')
        srow_s = srow_all[m][order]
        srow_s = np.where(is_hi[m][order], srow_s - HALF, srow_s)
        dl_s = dl[m][order]
        counts = np.bincount(key, minlength=2 * NG)
        buckets.append((srow_s, dl_s, counts))

    # tapered chunks: small final chunks shrink the serial tail after the
    # last gather and let phase-boundary AllGathers fire earlier
    bounds = [0, 7, 14, 21, 28, 35, 41, 45, 47, NG]
    chunk_groups = [list(range(bounds[i], bounds[i + 1]))
                    for i in range(len(bounds) - 1)]

    # per (chunk, half): packed streams and union plan
    T_lo, T_hi, OFF_lo, OFF_hi = [], [], [], []
    entries = []                       # per chunk: list of (g, [(half,t,col)])
    gidx_flat = {0: [], 1: []}         # per half: list of [T*128] idx arrays/core
    dstl_cols = {0: [], 1: []}         # per half: list of [128] col arrays/core
    for c in range(NCORES):
        gidx_flat[0].append([])
        gidx_flat[1].append([])
        dstl_cols[0].append([])
        dstl_cols[1].append([])

    for ci, groups in enumerate(chunk_groups):
        ents_per_g = {g: [] for g in groups}
        for half in (0, 1):
            # per-core prefix offsets within the packed chunk stream
            pref = np.zeros((NCORES, len(groups) + 1), np.int64)
            for c in range(NCORES):
                cnts = [buckets[c][2][g * 2 + half] for g in groups]
                pref[c, 1:] = np.cumsum(cnts)
            T_ch = int(max(1, -(-pref[:, -1].max() // P)))
            if half == 0:
                OFF_lo.append(sum(T_lo)); T_lo.append(T_ch)
            else:
                OFF_hi.append(sum(T_hi)); T_hi.append(T_ch)
            # per-core packed idx stream
            for c in range(NCORES):
                srow_s, dl_s, counts = buckets[c]
                starts = np.concatenate([[0], np.cumsum(counts)[:-1]])
                stream_idx = np.zeros(T_ch * P, np.int64)
                stream_dl = np.full(T_ch * P, -1, np.int16)
                stream_g = np.full(T_ch * P, -1, np.int64)
                for gi_, g in enumerate(groups):
                    b0 = starts[g * 2 + half]
                    n = counts[g * 2 + half]
                    p0 = pref[c, gi_]
                    stream_idx[p0:p0 + n] = srow_s[b0:b0 + n]
                    stream_dl[p0:p0 + n] = dl_s[b0:b0 + n]
                    stream_g[p0:p0 + n] = g
                gidx_flat[half][c].append(stream_idx)
                dstl_cols[half][c].append((stream_dl, stream_g))
            # union plan: (tile, group) pairs
            for gi_, g in enumerate(groups):
                tiles = set()
                for c in range(NCORES):
                    n = pref[c, gi_ + 1] - pref[c, gi_]
                    if n > 0:
                        t0 = pref[c, gi_] // P
                        t1 = (pref[c, gi_ + 1] - 1) // P
                        tiles.update(range(int(t0), int(t1) + 1))
                for t in sorted(tiles):
                    ents_per_g[g].append((half, t))
        ents_list = []
        for g in groups:
            ents_list.append((g, ents_per_g[g]))
        entries.append(ents_list)

    # assign global one-hot ids in emission order
    NE_tot = 0
    entries_final = []
    for ci, ents_list in enumerate(entries):
        lst = []
        for g, ents in ents_list:
            ents2 = []
            if not ents:
                # dummy entry to zero the accumulator
                ents = [(0, 0)]
            for half, t in ents:
                ents2.append((half, t, NE_tot))
                NE_tot += 1
            lst.append((g, ents2))
        entries_final.append(lst)

    # build one-hot column data [NCORES, 128, NE_tot] then expand to one-hot
    # matrices [NCORES, 128, NE_tot*128] fp16 (rhs stream, emission order)
    colv_arr = np.full((NCORES, 128, NE_tot), -1, np.int16)
    for ci, lst in enumerate(entries_final):
        for g, ents2 in lst:
            for half, t, eid in ents2:
                for c in range(NCORES):
                    stream_dl, stream_g = dstl_cols[half][c][ci]
                    seg_dl = stream_dl[t * P:(t + 1) * P]
                    seg_g = stream_g[t * P:(t + 1) * P]
                    colv_arr[c, :, eid] = np.where(seg_g == g, seg_dl, -1)

    import ml_dtypes
    ohs_arr = np.zeros((NCORES, 128, NE_tot, 128), ml_dtypes.float8_e4m3)
    cv = colv_arr.astype(np.int64)           # [C,128,NE]
    valid = cv >= 0
    np.put_along_axis(ohs_arr, np.maximum(cv, 0)[..., None],
                      valid[..., None].astype(ml_dtypes.float8_e4m3), axis=3)
    ohs_arr = ohs_arr.reshape(NCORES, 128, NE_tot * 128)

    plan = {
        "T_lo": tuple(T_lo), "T_hi": tuple(T_hi),
        "OFF_lo": tuple(OFF_lo), "OFF_hi": tuple(OFF_hi),
        "entries": entries_final,
        "NE_tot": NE_tot,
    }

    A = np.stack([W1[0] - W1[2], -W1[1], 2.0 * W1[2]])
    B = np.stack([W2[0] - W2[2], -W2[1], 2.0 * W2[2]])
    Acat = np.concatenate([A[0], A[1], A[2]], axis=1).astype(np.float32)
    Bcat = np.concatenate([B[0], B[1], B[2]], axis=1).astype(np.float32)
    b1_col = np.zeros((128, 1), np.float32)
    b1_col[:, 0] = b1
    b2_col = np.zeros((128, 1), np.float32)
    b2_col[:NCLS, 0] = b2

    def wrap_idx(a):
        w = a.reshape(-1, 16).T.astype(np.int16)
        return np.ascontiguousarray(np.tile(w, (8, 1)))

    # full phase-1 table (chunk-concat layout), host-computed: dinv * x
    xp = np.zeros((NCORES, NLOCP, D), np.float32)
    dp = np.zeros((NCORES, NLOCP), np.float32)
    for c in range(NCORES):
        xp[c, :NLOC] = x[c * NLOC:(c + 1) * NLOC]
        dp[c, :NLOC] = dinv[c * NLOC:(c + 1) * NLOC]
    t0_all = (xp * dp[:, :, None]).astype(np.float16)   # [C, NLOCP, D]
    tbl0 = np.zeros((VPAD, D), np.float16)
    for j, (g0, g1) in enumerate(CHUNK_G):
        blk = t0_all[:, g0 * P:g1 * P, :]               # [C, rows_j, D]
        tbl0[CHUNK_B[j]:CHUNK_B[j] + NCORES * CHUNK_ROWS[j]] = \
            blk.reshape(NCORES * CHUNK_ROWS[j], D)

    in_maps = []
    for c in range(NCORES):
        glo_flat = np.concatenate(gidx_flat[0][c])
        ghi_flat = np.concatenate(gidx_flat[1][c])

        x_c = np.zeros((NLOCP, D), np.float32)
        x_c[:NLOC] = x[c * NLOC:(c + 1) * NLOC]
        dinv_c = np.zeros(NLOCP, np.float32)
        dinv_c[:NLOC] = dinv[c * NLOC:(c + 1) * NLOC]
        dinvb = np.ascontiguousarray(
            np.broadcast_to(dinv_c[None, :], (128, NLOCP))).astype(np.float32)
        xT_c = np.ascontiguousarray(x_c.T).astype(np.float16)

        in_maps.append({
            "tbl0": tbl0,
            "xT": xT_c,
            "gidx_lo": wrap_idx(glo_flat),
            "gidx_hi": wrap_idx(ghi_flat),
            "ohs": np.ascontiguousarray(ohs_arr[c]),
            "dinvb": dinvb,
            "Acat": Acat.astype(np.float16),
            "Bcat": Bcat.astype(np.float16),
            "b1": b1_col,
            "b2": b2_col,
        })
    return in_maps, plan


def _plan_key(plan):
    ek = tuple((g, tuple(e)) for lst in plan["entries"] for g, e in lst)
    return (plan["T_lo"], plan["T_hi"], ek)


def kernel(x, edge_index, W1, b1, W2, b2, _trace=False, _tmpdir=None):
    from concourse.bass_utils import run_bass_kernel_spmd

    in_maps, plan = _host_prep(x, edge_index, W1, b1, W2, b2)
    key = _plan_key(plan)
    if key not in _prog_cache:
        _prog_cache[key] = _build_program(plan)
    nc = _prog_cache[key]

    res = run_bass_kernel_spmd(nc, in_maps, list(range(NCORES)),
                               trace=_trace, tmpdir=_tmpdir)
    out = np.concatenate([res.results[c]["out"] for c in range(NCORES)], axis=0)
    kernel._last_results = res
    return out

